# revision 23
# baseline (speedup 1.0000x reference)
"""Trainium2 Bass kernel for nn_Baseline_node2vec.

Computation (per pair e): logits[e] = relu(concat(embs[i_e], embs[j_e]) @ W1 + b1) @ W2 + b2

Strategy (data-parallel over the E=1M pairs, 8 cores, ~125k pairs/core):
  - Gather embedding rows with the ANT dma_gather extended instruction
    (int16 indices, 1024 rows per instruction, 4 SWDGE queues). The int16
    range only addresses 32768 rows, so the 100k-row table is viewed as 4
    windows of 25000 rows; the host buckets pairs into 16 (Lwindow,Rwindow)
    groups, pads each group to a 1024-pair multiple, and the device program
    is built for that (cached) schedule.
  - dma_gather lands rows as [pair%128 -> partition, pair//128 -> block]:
    chunks of 128 consecutive pairs, pairs-on-partition. PE transposes each
    [128,128] chunk (fp32r) into feat-on-partition layout, then fp32r
    matmuls: hT = W1.T @ xT (relu+bias on ACT), logitsT = W2.T @ hT.
  - Output is produced as [2, E_pad] channel-major; the host scatters it
    back to the original pair order.
"""

import numpy as np

import concourse.bacc as bacc
import concourse.mybir as mybir
import concourse.tile as tile
from concourse import bass_utils
from concourse.bass_interp import get_hw_module
from concourse.library_config import mlp

N_NODES = 100000
D = 128
HID = 256
E_TOTAL = 1000000
N_CORES = 8
E_CORE = E_TOTAL // N_CORES            # 125000
W = 25000                              # int16-addressable table window
NW = 4
GBP = 1024                             # pairs per gather block
NB = 512                               # pairs per compute block
NB_PER_GB = GBP // NB                  # 2
CHUNK = 128

f32 = mybir.dt.float32
f32r = mybir.dt.float32r
f16 = mybir.dt.float16
i32 = mybir.dt.int32
i16 = mybir.dt.int16
RELU = mybir.ActivationFunctionType.Relu
IDENT = mybir.ActivationFunctionType.Identity


def build_program(schedule, num_devices=N_CORES):
    """schedule: tuple of (wL, wR, npairs) per gather block (npairs 1024|512)."""
    n_gb = len(schedule)
    e_pad = sum(sz for _, _, sz, _ in schedule)
    nc = bacc.Bacc(
        "TRN2",
        target_bir_lowering=False,
        debug=False,
        enable_asserts=False,
        num_devices=num_devices,
        num_swdge_queues=4,
    )

    embs = nc.dram_tensor("embs", [N_NODES, D], f16, kind="ExternalInput").ap()
    # per GB: 64 cols of wrapped L idx + 64 cols of wrapped R idx
    n_idx_cols = sum(sz // 8 for _, _, sz, _ in schedule)
    idxT = nc.dram_tensor("idxT", [128, n_idx_cols], i16, kind="ExternalInput").ap()
    w1 = nc.dram_tensor("w1", [2 * D, HID], f16, kind="ExternalInput").ap()
    b1v = nc.dram_tensor("b1v", [128, 2], f32, kind="ExternalInput").ap()
    w2 = nc.dram_tensor("w2", [HID, 2], f16, kind="ExternalInput").ap()
    b2v = nc.dram_tensor("b2v", [2, 1], f32, kind="ExternalInput").ap()
    idn = nc.dram_tensor("idn", [128, 128], f16, kind="ExternalInput").ap()
    outT = nc.dram_tensor("outT", [2, e_pad], f32, kind="ExternalOutput").ap()

    with tile.TileContext(nc) as tc:
        with (
            tc.tile_pool(name="consts", bufs=1) as cpool,
            tc.tile_pool(name="gbuf", bufs=6) as gpool,
            tc.tile_pool(name="xt", bufs=6) as xpool,
            tc.tile_pool(name="ht", bufs=4) as hpool,
            tc.tile_pool(name="ob", bufs=2) as opool,
            tc.tile_pool(name="ps_x", bufs=3, space="PSUM") as ps_x,
            tc.tile_pool(name="ps_h", bufs=3, space="PSUM") as ps_h,
            tc.tile_pool(name="ps_l", bufs=2, space="PSUM") as ps_l,
        ):
            nc.gpsimd.load_library(mlp)
            ident = cpool.tile([128, 128], f16, name="ident")
            nc.sync.dma_start(out=ident[:], in_=idn[:, :])
            w1_sb = cpool.tile([128, 512], f16, name="w1_sb")
            nc.sync.dma_start(out=w1_sb[:, 0:256], in_=w1[0:128, :])
            nc.sync.dma_start(out=w1_sb[:, 256:512], in_=w1[128:256, :])
            w2_sb = cpool.tile([128, 4], f16, name="w2_sb")
            nc.sync.dma_start(out=w2_sb[:, 0:2], in_=w2[0:128, :])
            nc.sync.dma_start(out=w2_sb[:, 2:4], in_=w2[128:256, :])
            b1_sb = cpool.tile([128, 2], f32, name="b1_sb")
            nc.sync.dma_start(out=b1_sb[:], in_=b1v[:, :])
            b2_sb = cpool.tile([2, 1], f32, name="b2_sb")
            nc.sync.dma_start(out=b2_sb[:], in_=b2v[:, :])
            col_off = [0]
            for _, _, sz, _ in schedule:
                col_off.append(col_off[-1] + sz // 8)
            head_cols = col_off[min(4, n_gb)]
            idx_sbA = cpool.tile([128, head_cols], i16, name="idx_sbA")
            nc.sync.dma_start(out=idx_sbA[:], in_=idxT[:, :head_cols])
            idx_sbB = cpool.tile([128, max(1, n_idx_cols - head_cols)], i16, name="idx_sbB")
            if n_idx_cols > head_cols:
                nc.sync.dma_start(out=idx_sbB[:], in_=idxT[:, head_cols:])

            for gb, (wl, wr, sz, layoff) in enumerate(schedule):
                nblk = (sz + 127) // 128             # compute chunks (ceil)
                half = sz // 16                      # idx cols per side
                c0 = col_off[gb] - (0 if gb < 4 else head_cols)
                isb = idx_sbA if gb < 4 else idx_sbB
                gl = gpool.tile([128, 8 * CHUNK], f16, name="gl", tag="gl")
                gr = gpool.tile([128, 8 * CHUNK], f16, name="gr", tag="gr")
                nc.gpsimd.dma_gather(
                    out_ap=gl[:, :nblk * D].rearrange("p (b d) -> p b d", d=D),
                    in_ap=embs[wl * W:(wl + 1) * W, :],
                    idxs_ap=isb[:, c0:c0 + half],
                    num_idxs=sz, num_idxs_reg=sz, elem_size=D,
                    queue_num=(2 * gb) % 4,
                )
                nc.gpsimd.dma_gather(
                    out_ap=gr[:, :nblk * D].rearrange("p (b d) -> p b d", d=D),
                    in_ap=embs[wr * W:(wr + 1) * W, :],
                    idxs_ap=isb[:, c0 + half:c0 + 2 * half],
                    num_idxs=sz, num_idxs_reg=sz, elem_size=D,
                    queue_num=(2 * gb + 1) % 4,
                )
                ob = opool.tile([2, GBP], f32, name="ob", tag="ob")
                # v-blocks of up to 512 pairs; the tail block may be any
                # multiple of 128 (block sizes are 128-granular)
                vszs = []
                rem_chunks = nblk
                while rem_chunks > 0:
                    take = min(4, rem_chunks)
                    vszs.append(take * CHUNK)
                    rem_chunks -= take
                n_v = len(vszs)
                voff = [0]
                for vsz in vszs:
                    voff.append(voff[-1] + vsz)
                # transposes for all compute blocks, then W1 matmuls grouped
                # by stationary operand so each W1 chunk is loaded once per GB
                xts = []   # (xtl, xtr) per v
                for v, vsz in enumerate(vszs):
                    xtl_ps = ps_x.tile([128, NB], f16, name="xtl_ps", tag="psx")
                    xtr_ps = ps_x.tile([128, NB], f16, name="xtr_ps", tag="psx")
                    for s in range(vsz // CHUNK):
                        c = voff[v] + s * CHUNK
                        nc.tensor.transpose(
                            out=xtl_ps[:, s * 128:(s + 1) * 128],
                            in_=gl[:, c:c + 128],
                            identity=ident[:],
                        )
                        nc.tensor.transpose(
                            out=xtr_ps[:, s * 128:(s + 1) * 128],
                            in_=gr[:, c:c + 128],
                            identity=ident[:],
                        )
                    xtl = xpool.tile([128, NB], f16, name="xtl", tag="xt")
                    xtr = xpool.tile([128, NB], f16, name="xtr", tag="xt")
                    nc.vector.tensor_copy(xtl[:, :vsz], xtl_ps[:, :vsz])
                    nc.vector.tensor_copy(xtr[:, :vsz], xtr_ps[:, :vsz])
                    xts.append((xtl, xtr))

                hs = [(ps_h.tile([128, NB], f32, name=f"h0v{v}", tag="psh"),
                       ps_h.tile([128, NB], f32, name=f"h1v{v}", tag="psh"))
                      for v in range(n_v)]
                # W1 matmuls grouped by stationary operand across v
                for v, vsz in enumerate(vszs):
                    nc.tensor.matmul(hs[v][0][:, :vsz], w1_sb[:, 0:128],
                                     xts[v][0][:, :vsz], start=True, stop=False)
                for v, vsz in enumerate(vszs):
                    nc.tensor.matmul(hs[v][0][:, :vsz], w1_sb[:, 256:384],
                                     xts[v][1][:, :vsz], start=False, stop=True)
                for v, vsz in enumerate(vszs):
                    nc.tensor.matmul(hs[v][1][:, :vsz], w1_sb[:, 128:256],
                                     xts[v][0][:, :vsz], start=True, stop=False)
                for v, vsz in enumerate(vszs):
                    nc.tensor.matmul(hs[v][1][:, :vsz], w1_sb[:, 384:512],
                                     xts[v][1][:, :vsz], start=False, stop=True)

                for v, vsz in enumerate(vszs):
                    ht0 = hpool.tile([128, NB], f16, name="ht0", tag="ht")
                    ht1 = hpool.tile([128, NB], f16, name="ht1", tag="ht")
                    nc.scalar.activation(ht0[:, :vsz], hs[v][0][:, :vsz], RELU,
                                         bias=b1_sb[:, 0:1], scale=1.0)
                    nc.scalar.activation(ht1[:, :vsz], hs[v][1][:, :vsz], RELU,
                                         bias=b1_sb[:, 1:2], scale=1.0)
                    lps = ps_l.tile([2, NB], f32, name="lps", tag="psl")
                    nc.tensor.matmul(lps[:, :vsz], w2_sb[:, 0:2], ht0[:, :vsz],
                                     start=True, stop=False)
                    nc.tensor.matmul(lps[:, :vsz], w2_sb[:, 2:4], ht1[:, :vsz],
                                     start=False, stop=True)
                    nc.vector.tensor_tensor(
                        out=ob[:, voff[v]:voff[v] + vsz], in0=lps[:, :vsz],
                        in1=b2_sb[:, 0:1].to_broadcast([2, vsz]),
                        op=mybir.AluOpType.add,
                    )
                nc.sync.dma_start(
                    out=outT[:, layoff:layoff + sz], in_=ob[:, :sz],
                )

    nc.compile()
    return nc


def plan_schedule(idx_all_i32):
    """Global plan: bucket ALL pairs by (windowL, windowR), deal each group
    evenly across the 8 cores, pad per-core group sizes to 16-granular
    gathers. Blocks carry their layout offset; the small remainder blocks
    are emitted last (smallest at the very end) so the end-of-stream
    compute backlog drains fast. Returns (schedule, S, bal, key)."""
    key = (idx_all_i32[:, 0] // W) * 4 + (idx_all_i32[:, 1] // W)
    gtot = np.bincount(key, minlength=16)
    bal = (gtot + N_CORES - 1) // N_CORES          # per-core share (ceil)
    S = ((bal + 15) // 16) * 16                    # 16-granular padded size
    starts = np.zeros(17, np.int64)
    starts[1:] = np.cumsum(S)
    fulls, rems = [], []
    for g in range(16):
        full, rem = divmod(int(S[g]), GBP)
        for k in range(full):
            fulls.append((g // 4, g % 4, GBP, int(starts[g]) + k * GBP))
        if rem:
            rems.append((g // 4, g % 4, rem, int(starts[g]) + full * GBP))
    # small remainder blocks go FIRST: early in the stream the gather is the
    # limiter and PE has spare capacity to absorb their lower efficiency;
    # ending on full 1024-blocks keeps the final compute backlog shallow
    rems.sort(key=lambda b: b[2])
    return tuple(rems + fulls), S, bal, key


def prepare_core(share, S, schedule):
    """share: list of 16 arrays of [n_g, 2] node-index pairs (this core's
    share of each group). Builds the wrapped idx tensor; real pairs sit at
    the front of each group's padded span."""
    e_pad = int(S.sum())
    starts = np.zeros(17, np.int64)
    starts[1:] = np.cumsum(S)
    L = np.empty(e_pad, np.int32)
    R = np.empty(e_pad, np.int32)
    for g in range(16):
        L[starts[g]:starts[g + 1]] = (g // 4) * W
        R[starts[g]:starts[g + 1]] = (g % 4) * W
        n = len(share[g])
        L[starts[g]:starts[g] + n] = share[g][:, 0]
        R[starts[g]:starts[g] + n] = share[g][:, 1]
    L16 = (L - (L // W) * W).astype(np.int16)
    R16 = (R - (R // W) * W).astype(np.int16)
    # idx columns follow the schedule's block emission order; each block
    # pulls its pairs from its layout offset
    total_cols = sum(sz // 8 for _, _, sz, _ in schedule)
    cols = np.empty((128, total_cols), np.int16)
    c_off = 0
    for _, _, sz, layoff in schedule:
        for arr in (L16, R16):
            seg = arr[layoff:layoff + sz]
            wt = seg.reshape(sz // 16, 16).T       # [16, sz/16]
            cols[:, c_off:c_off + sz // 16] = np.tile(wt, (8, 1))
            c_off += sz // 16
    return np.ascontiguousarray(cols)


_CACHE = {}


def _get_program(schedule):
    if _CACHE.get("schedule") != schedule:
        _CACHE["nc"] = build_program(schedule)
        _CACHE["schedule"] = schedule
    return _CACHE["nc"]


def run_on_hw(nc, in_maps, trace=False, **kw):
    old = nc.m
    nc.m = get_hw_module(nc.m)
    try:
        return bass_utils.run_bass_kernel_spmd(
            nc, in_maps, core_ids=list(range(len(in_maps))), trace=trace, **kw
        )
    finally:
        nc.m = old


def make_in_maps(spatial_nodes_embs, node_indices, W1, b1, W2, b2):
    embs = np.ascontiguousarray(np.asarray(spatial_nodes_embs), dtype=np.float16)
    idx = np.asarray(node_indices).astype(np.int32)
    w1 = np.ascontiguousarray(np.asarray(W1), dtype=np.float16)
    b1 = np.asarray(b1, dtype=np.float32)
    w2 = np.ascontiguousarray(np.asarray(W2), dtype=np.float16)
    b2 = np.asarray(b2, dtype=np.float32)
    b1v = np.ascontiguousarray(b1.reshape(2, 128).T)
    b2v = np.ascontiguousarray(b2.reshape(2, 1))
    idn = np.eye(128, dtype=np.float16)
    schedule, S, bal, key = plan_schedule(idx)
    starts = np.zeros(17, np.int64)
    starts[1:] = np.cumsum(S)
    group_members = [np.flatnonzero(key == g) for g in range(16)]
    in_maps, origs, poss = [], [], []
    for c in range(N_CORES):
        share, orig, pos = [], [], []
        for g in range(16):
            mem = group_members[g][c * bal[g]:(c + 1) * bal[g]]
            share.append(idx[mem])
            orig.append(mem)
            pos.append(starts[g] + np.arange(len(mem)))
        cols = prepare_core(share, S, schedule)
        origs.append(np.concatenate(orig))
        poss.append(np.concatenate(pos))
        in_maps.append({
            "embs": embs, "idxT": cols, "w1": w1, "b1v": b1v,
            "w2": w2, "b2v": b2v, "idn": idn,
        })
    return schedule, in_maps, origs, poss


def kernel(spatial_nodes_embs, node_indices, W1, b1, W2, b2):
    schedule, in_maps, origs, poss = make_in_maps(
        spatial_nodes_embs, node_indices, W1, b1, W2, b2)
    nc = _get_program(schedule)
    res = run_on_hw(nc, in_maps)
    out = np.empty((E_TOTAL, 2), dtype=np.float32)
    for c in range(N_CORES):
        oT = res.results[c]["outT"]              # [2, e_pad]
        out[origs[c]] = oT[:, poss[c]].T         # scatter to original order
    return out



# revision 24
# speedup vs baseline: 1.1895x; 1.1895x over previous
"""Trainium2 Bass kernel for nn_Baseline_node2vec.

Computation (per pair e): logits[e] = relu(concat(embs[i_e], embs[j_e]) @ W1 + b1) @ W2 + b2

Strategy (data-parallel over the E=1M pairs, 8 cores, ~125k pairs/core):
  - Gather embedding rows with the ANT dma_gather extended instruction
    (int16 indices, 1024 rows per instruction, 4 SWDGE queues). The int16
    range only addresses 32768 rows, so the 100k-row table is viewed as 4
    windows of 25000 rows; the host buckets pairs into 16 (Lwindow,Rwindow)
    groups, pads each group to a 1024-pair multiple, and the device program
    is built for that (cached) schedule.
  - dma_gather lands rows as [pair%128 -> partition, pair//128 -> block]:
    chunks of 128 consecutive pairs, pairs-on-partition. PE transposes each
    [128,128] chunk (fp32r) into feat-on-partition layout, then fp32r
    matmuls: hT = W1.T @ xT (relu+bias on ACT), logitsT = W2.T @ hT.
  - Output is produced as [2, E_pad] channel-major; the host scatters it
    back to the original pair order.
"""

import numpy as np

import concourse.bacc as bacc
import concourse.mybir as mybir
import concourse.tile as tile
from concourse import bass_utils
from concourse.bass_interp import get_hw_module
from concourse.library_config import mlp

N_NODES = 100000
D = 128
HID = 256
E_TOTAL = 1000000
N_CORES = 8
E_CORE = E_TOTAL // N_CORES            # 125000
W = 25000                              # int16-addressable table window
NW = 4
GBP = 1024                             # pairs per gather block
NB = 512                               # pairs per compute block
NB_PER_GB = GBP // NB                  # 2
CHUNK = 128

f32 = mybir.dt.float32
f32r = mybir.dt.float32r
f16 = mybir.dt.float16
i32 = mybir.dt.int32
i16 = mybir.dt.int16
RELU = mybir.ActivationFunctionType.Relu
IDENT = mybir.ActivationFunctionType.Identity


def build_program(schedule, num_devices=N_CORES):
    """schedule: tuple of (wL, wR, npairs) per gather block (npairs 1024|512)."""
    n_gb = len(schedule)
    e_pad = sum(sz for _, _, sz, _ in schedule)
    nc = bacc.Bacc(
        "TRN2",
        target_bir_lowering=False,
        debug=False,
        enable_asserts=False,
        num_devices=num_devices,
        num_swdge_queues=4,
    )

    embs = nc.dram_tensor("embs", [N_NODES, D], f16, kind="ExternalInput").ap()
    # per GB: 64 cols of wrapped L idx + 64 cols of wrapped R idx
    n_idx_cols = sum(sz // 8 for _, _, sz, _ in schedule)
    idxT = nc.dram_tensor("idxT", [128, n_idx_cols], i16, kind="ExternalInput").ap()
    w1 = nc.dram_tensor("w1", [2 * D, HID], f16, kind="ExternalInput").ap()
    b1v = nc.dram_tensor("b1v", [128, 2], f32, kind="ExternalInput").ap()
    w2 = nc.dram_tensor("w2", [HID, 2], f16, kind="ExternalInput").ap()
    b2v = nc.dram_tensor("b2v", [2, 1], f32, kind="ExternalInput").ap()
    idn = nc.dram_tensor("idn", [128, 128], f16, kind="ExternalInput").ap()
    outT = nc.dram_tensor("outT", [2, e_pad], f32, kind="ExternalOutput").ap()

    with tile.TileContext(nc) as tc:
        with (
            tc.tile_pool(name="consts", bufs=1) as cpool,
            tc.tile_pool(name="gbuf", bufs=8) as gpool,
            tc.tile_pool(name="xt", bufs=6) as xpool,
            tc.tile_pool(name="ht", bufs=4) as hpool,
            tc.tile_pool(name="ob", bufs=2) as opool,
            tc.tile_pool(name="ps_x", bufs=3, space="PSUM") as ps_x,
            tc.tile_pool(name="ps_h", bufs=3, space="PSUM") as ps_h,
            tc.tile_pool(name="ps_l", bufs=2, space="PSUM") as ps_l,
        ):
            nc.gpsimd.load_library(mlp)
            ident = cpool.tile([128, 128], f16, name="ident")
            nc.sync.dma_start(out=ident[:], in_=idn[:, :])
            w1_sb = cpool.tile([128, 512], f16, name="w1_sb")
            nc.sync.dma_start(out=w1_sb[:, 0:256], in_=w1[0:128, :])
            nc.sync.dma_start(out=w1_sb[:, 256:512], in_=w1[128:256, :])
            w2_sb = cpool.tile([128, 4], f16, name="w2_sb")
            nc.sync.dma_start(out=w2_sb[:, 0:2], in_=w2[0:128, :])
            nc.sync.dma_start(out=w2_sb[:, 2:4], in_=w2[128:256, :])
            b1_sb = cpool.tile([128, 2], f32, name="b1_sb")
            nc.sync.dma_start(out=b1_sb[:], in_=b1v[:, :])
            b2_sb = cpool.tile([2, 1], f32, name="b2_sb")
            nc.sync.dma_start(out=b2_sb[:], in_=b2v[:, :])
            col_off = [0]
            for _, _, sz, _ in schedule:
                col_off.append(col_off[-1] + sz // 8)
            head_cols = col_off[min(4, n_gb)]
            idx_sbA = cpool.tile([128, head_cols], i16, name="idx_sbA")
            nc.sync.dma_start(out=idx_sbA[:], in_=idxT[:, :head_cols])
            idx_sbB = cpool.tile([128, max(1, n_idx_cols - head_cols)], i16, name="idx_sbB")
            if n_idx_cols > head_cols:
                nc.sync.dma_start(out=idx_sbB[:], in_=idxT[:, head_cols:])

            for gb, (wl, wr, sz, layoff) in enumerate(schedule):
                nblk = (sz + 127) // 128             # compute chunks (ceil)
                half = sz // 16                      # idx cols per side
                c0 = col_off[gb] - (0 if gb < 4 else head_cols)
                isb = idx_sbA if gb < 4 else idx_sbB
                gl = gpool.tile([128, 8 * CHUNK], f16, name="gl", tag="gl")
                gr = gpool.tile([128, 8 * CHUNK], f16, name="gr", tag="gr")
                nc.gpsimd.dma_gather(
                    out_ap=gl[:, :nblk * D].rearrange("p (b d) -> p b d", d=D),
                    in_ap=embs[wl * W:(wl + 1) * W, :],
                    idxs_ap=isb[:, c0:c0 + half],
                    num_idxs=sz, num_idxs_reg=sz, elem_size=D,
                    queue_num=(2 * gb) % 4,
                )
                nc.gpsimd.dma_gather(
                    out_ap=gr[:, :nblk * D].rearrange("p (b d) -> p b d", d=D),
                    in_ap=embs[wr * W:(wr + 1) * W, :],
                    idxs_ap=isb[:, c0 + half:c0 + 2 * half],
                    num_idxs=sz, num_idxs_reg=sz, elem_size=D,
                    queue_num=(2 * gb + 1) % 4,
                )
                ob = opool.tile([2, GBP], f32, name="ob", tag="ob")
                # v-blocks of up to 512 pairs; the tail block may be any
                # multiple of 128 (block sizes are 128-granular)
                vszs = []
                rem_chunks = nblk
                while rem_chunks > 0:
                    take = min(4, rem_chunks)
                    vszs.append(take * CHUNK)
                    rem_chunks -= take
                n_v = len(vszs)
                voff = [0]
                for vsz in vszs:
                    voff.append(voff[-1] + vsz)
                # transposes for all compute blocks, then W1 matmuls grouped
                # by stationary operand so each W1 chunk is loaded once per GB
                xts = []   # (xtl, xtr) per v
                for v, vsz in enumerate(vszs):
                    xtl_ps = ps_x.tile([128, NB], f16, name="xtl_ps", tag="psx")
                    xtr_ps = ps_x.tile([128, NB], f16, name="xtr_ps", tag="psx")
                    for s in range(vsz // CHUNK):
                        c = voff[v] + s * CHUNK
                        nc.tensor.transpose(
                            out=xtl_ps[:, s * 128:(s + 1) * 128],
                            in_=gl[:, c:c + 128],
                            identity=ident[:],
                        )
                        nc.tensor.transpose(
                            out=xtr_ps[:, s * 128:(s + 1) * 128],
                            in_=gr[:, c:c + 128],
                            identity=ident[:],
                        )
                    xtl = xpool.tile([128, NB], f16, name="xtl", tag="xt")
                    xtr = xpool.tile([128, NB], f16, name="xtr", tag="xt")
                    nc.vector.tensor_copy(xtl[:, :vsz], xtl_ps[:, :vsz])
                    nc.vector.tensor_copy(xtr[:, :vsz], xtr_ps[:, :vsz])
                    xts.append((xtl, xtr))

                hs = [(ps_h.tile([128, NB], f32, name=f"h0v{v}", tag="psh"),
                       ps_h.tile([128, NB], f32, name=f"h1v{v}", tag="psh"))
                      for v in range(n_v)]
                # W1 matmuls grouped by stationary operand across v
                for v, vsz in enumerate(vszs):
                    nc.tensor.matmul(hs[v][0][:, :vsz], w1_sb[:, 0:128],
                                     xts[v][0][:, :vsz], start=True, stop=False)
                for v, vsz in enumerate(vszs):
                    nc.tensor.matmul(hs[v][0][:, :vsz], w1_sb[:, 256:384],
                                     xts[v][1][:, :vsz], start=False, stop=True)
                for v, vsz in enumerate(vszs):
                    nc.tensor.matmul(hs[v][1][:, :vsz], w1_sb[:, 128:256],
                                     xts[v][0][:, :vsz], start=True, stop=False)
                for v, vsz in enumerate(vszs):
                    nc.tensor.matmul(hs[v][1][:, :vsz], w1_sb[:, 384:512],
                                     xts[v][1][:, :vsz], start=False, stop=True)

                for v, vsz in enumerate(vszs):
                    ht0 = hpool.tile([128, NB], f16, name="ht0", tag="ht")
                    ht1 = hpool.tile([128, NB], f16, name="ht1", tag="ht")
                    nc.scalar.activation(ht0[:, :vsz], hs[v][0][:, :vsz], RELU,
                                         bias=b1_sb[:, 0:1], scale=1.0)
                    nc.scalar.activation(ht1[:, :vsz], hs[v][1][:, :vsz], RELU,
                                         bias=b1_sb[:, 1:2], scale=1.0)
                    lps = ps_l.tile([2, NB], f32, name="lps", tag="psl")
                    nc.tensor.matmul(lps[:, :vsz], w2_sb[:, 0:2], ht0[:, :vsz],
                                     start=True, stop=False)
                    nc.tensor.matmul(lps[:, :vsz], w2_sb[:, 2:4], ht1[:, :vsz],
                                     start=False, stop=True)
                    nc.vector.tensor_tensor(
                        out=ob[:, voff[v]:voff[v] + vsz], in0=lps[:, :vsz],
                        in1=b2_sb[:, 0:1].to_broadcast([2, vsz]),
                        op=mybir.AluOpType.add,
                    )
                nc.sync.dma_start(
                    out=outT[:, layoff:layoff + sz], in_=ob[:, :sz],
                )

    nc.compile()
    return nc


def plan_schedule(idx_all_i32):
    """Global plan: bucket ALL pairs by (windowL, windowR), deal each group
    evenly across the 8 cores, pad per-core group sizes to 16-granular
    gathers. Blocks carry their layout offset; the small remainder blocks
    are emitted last (smallest at the very end) so the end-of-stream
    compute backlog drains fast. Returns (schedule, S, bal, key)."""
    key = (idx_all_i32[:, 0] // W) * 4 + (idx_all_i32[:, 1] // W)
    gtot = np.bincount(key, minlength=16)
    bal = (gtot + N_CORES - 1) // N_CORES          # per-core share (ceil)
    S = ((bal + 15) // 16) * 16                    # 16-granular padded size
    starts = np.zeros(17, np.int64)
    starts[1:] = np.cumsum(S)
    fulls, rems = [], []
    for g in range(16):
        full, rem = divmod(int(S[g]), GBP)
        for k in range(full):
            fulls.append((g // 4, g % 4, GBP, int(starts[g]) + k * GBP))
        if rem:
            rems.append((g // 4, g % 4, rem, int(starts[g]) + full * GBP))
    # small remainder blocks go FIRST: early in the stream the gather is the
    # limiter and PE has spare capacity to absorb their lower efficiency;
    # ending on full 1024-blocks keeps the final compute backlog shallow
    rems.sort(key=lambda b: b[2])
    return tuple(rems + fulls), S, bal, key


def prepare_core(share, S, schedule):
    """share: list of 16 arrays of [n_g, 2] node-index pairs (this core's
    share of each group). Builds the wrapped idx tensor; real pairs sit at
    the front of each group's padded span."""
    e_pad = int(S.sum())
    starts = np.zeros(17, np.int64)
    starts[1:] = np.cumsum(S)
    L = np.empty(e_pad, np.int32)
    R = np.empty(e_pad, np.int32)
    for g in range(16):
        L[starts[g]:starts[g + 1]] = (g // 4) * W
        R[starts[g]:starts[g + 1]] = (g % 4) * W
        n = len(share[g])
        L[starts[g]:starts[g] + n] = share[g][:, 0]
        R[starts[g]:starts[g] + n] = share[g][:, 1]
    L16 = (L - (L // W) * W).astype(np.int16)
    R16 = (R - (R // W) * W).astype(np.int16)
    # idx columns follow the schedule's block emission order; each block
    # pulls its pairs from its layout offset
    total_cols = sum(sz // 8 for _, _, sz, _ in schedule)
    cols = np.empty((128, total_cols), np.int16)
    c_off = 0
    for _, _, sz, layoff in schedule:
        for arr in (L16, R16):
            seg = arr[layoff:layoff + sz]
            wt = seg.reshape(sz // 16, 16).T       # [16, sz/16]
            cols[:, c_off:c_off + sz // 16] = np.tile(wt, (8, 1))
            c_off += sz // 16
    return np.ascontiguousarray(cols)


_CACHE = {}


def _get_program(schedule):
    if _CACHE.get("schedule") != schedule:
        _CACHE["nc"] = build_program(schedule)
        _CACHE["schedule"] = schedule
    return _CACHE["nc"]


def run_on_hw(nc, in_maps, trace=False, **kw):
    old = nc.m
    nc.m = get_hw_module(nc.m)
    try:
        return bass_utils.run_bass_kernel_spmd(
            nc, in_maps, core_ids=list(range(len(in_maps))), trace=trace, **kw
        )
    finally:
        nc.m = old


def make_in_maps(spatial_nodes_embs, node_indices, W1, b1, W2, b2):
    embs = np.ascontiguousarray(np.asarray(spatial_nodes_embs), dtype=np.float16)
    idx = np.asarray(node_indices).astype(np.int32)
    w1 = np.ascontiguousarray(np.asarray(W1), dtype=np.float16)
    b1 = np.asarray(b1, dtype=np.float32)
    w2 = np.ascontiguousarray(np.asarray(W2), dtype=np.float16)
    b2 = np.asarray(b2, dtype=np.float32)
    b1v = np.ascontiguousarray(b1.reshape(2, 128).T)
    b2v = np.ascontiguousarray(b2.reshape(2, 1))
    idn = np.eye(128, dtype=np.float16)
    schedule, S, bal, key = plan_schedule(idx)
    starts = np.zeros(17, np.int64)
    starts[1:] = np.cumsum(S)
    group_members = [np.flatnonzero(key == g) for g in range(16)]
    in_maps, origs, poss = [], [], []
    for c in range(N_CORES):
        share, orig, pos = [], [], []
        for g in range(16):
            mem = group_members[g][c * bal[g]:(c + 1) * bal[g]]
            share.append(idx[mem])
            orig.append(mem)
            pos.append(starts[g] + np.arange(len(mem)))
        cols = prepare_core(share, S, schedule)
        origs.append(np.concatenate(orig))
        poss.append(np.concatenate(pos))
        in_maps.append({
            "embs": embs, "idxT": cols, "w1": w1, "b1v": b1v,
            "w2": w2, "b2v": b2v, "idn": idn,
        })
    return schedule, in_maps, origs, poss


def kernel(spatial_nodes_embs, node_indices, W1, b1, W2, b2):
    schedule, in_maps, origs, poss = make_in_maps(
        spatial_nodes_embs, node_indices, W1, b1, W2, b2)
    nc = _get_program(schedule)
    res = run_on_hw(nc, in_maps)
    out = np.empty((E_TOTAL, 2), dtype=np.float32)
    for c in range(N_CORES):
        oT = res.results[c]["outT"]              # [2, e_pad]
        out[origs[c]] = oT[:, poss[c]].T         # scatter to original order
    return out



# revision 25
# speedup vs baseline: 1.1926x; 1.0027x over previous
"""Trainium2 Bass kernel for nn_Baseline_node2vec.

Computation (per pair e): logits[e] = relu(concat(embs[i_e], embs[j_e]) @ W1 + b1) @ W2 + b2

Strategy (data-parallel over the E=1M pairs, 8 cores, ~125k pairs/core):
  - Gather embedding rows with the ANT dma_gather extended instruction
    (int16 indices, 1024 rows per instruction, 4 SWDGE queues). The int16
    range only addresses 32768 rows, so the 100k-row table is viewed as 4
    windows of 25000 rows; the host buckets pairs into 16 (Lwindow,Rwindow)
    groups, pads each group to a 1024-pair multiple, and the device program
    is built for that (cached) schedule.
  - dma_gather lands rows as [pair%128 -> partition, pair//128 -> block]:
    chunks of 128 consecutive pairs, pairs-on-partition. PE transposes each
    [128,128] chunk (fp32r) into feat-on-partition layout, then fp32r
    matmuls: hT = W1.T @ xT (relu+bias on ACT), logitsT = W2.T @ hT.
  - Output is produced as [2, E_pad] channel-major; the host scatters it
    back to the original pair order.
"""

import numpy as np

import concourse.bacc as bacc
import concourse.mybir as mybir
import concourse.tile as tile
from concourse import bass_utils
from concourse.bass_interp import get_hw_module
from concourse.library_config import mlp

N_NODES = 100000
D = 128
HID = 256
E_TOTAL = 1000000
N_CORES = 8
E_CORE = E_TOTAL // N_CORES            # 125000
W = 25000                              # int16-addressable table window
NW = 4
GBP = 1024                             # pairs per gather block
NB = 512                               # pairs per compute block
NB_PER_GB = GBP // NB                  # 2
CHUNK = 128

f32 = mybir.dt.float32
f32r = mybir.dt.float32r
f16 = mybir.dt.float16
i32 = mybir.dt.int32
i16 = mybir.dt.int16
RELU = mybir.ActivationFunctionType.Relu
IDENT = mybir.ActivationFunctionType.Identity


def build_program(schedule, num_devices=N_CORES):
    """schedule: tuple of (wL, wR, npairs) per gather block (npairs 1024|512)."""
    n_gb = len(schedule)
    e_pad = sum(sz for _, _, sz, _ in schedule)
    nc = bacc.Bacc(
        "TRN2",
        target_bir_lowering=False,
        debug=False,
        enable_asserts=False,
        num_devices=num_devices,
        num_swdge_queues=4,
    )

    embs = nc.dram_tensor("embs", [N_NODES, D], f16, kind="ExternalInput").ap()
    # per GB: 64 cols of wrapped L idx + 64 cols of wrapped R idx
    n_idx_cols = sum(sz // 8 for _, _, sz, _ in schedule)
    idxT = nc.dram_tensor("idxT", [128, n_idx_cols], i16, kind="ExternalInput").ap()
    w1 = nc.dram_tensor("w1", [2 * D, HID], f16, kind="ExternalInput").ap()
    b1v = nc.dram_tensor("b1v", [128, 2], f32, kind="ExternalInput").ap()
    w2 = nc.dram_tensor("w2", [HID, 2], f16, kind="ExternalInput").ap()
    b2v = nc.dram_tensor("b2v", [2, 1], f32, kind="ExternalInput").ap()
    idn = nc.dram_tensor("idn", [128, 128], f16, kind="ExternalInput").ap()
    outT = nc.dram_tensor("outT", [2, e_pad], f32, kind="ExternalOutput").ap()

    with tile.TileContext(nc) as tc:
        with (
            tc.tile_pool(name="consts", bufs=1) as cpool,
            tc.tile_pool(name="gbuf", bufs=8) as gpool,
            tc.tile_pool(name="xt", bufs=6) as xpool,
            tc.tile_pool(name="ht", bufs=4) as hpool,
            tc.tile_pool(name="ob", bufs=2) as opool,
            tc.tile_pool(name="ps_x", bufs=3, space="PSUM") as ps_x,
            tc.tile_pool(name="ps_h", bufs=3, space="PSUM") as ps_h,
            tc.tile_pool(name="ps_l", bufs=2, space="PSUM") as ps_l,
        ):
            nc.gpsimd.load_library(mlp)
            # first idx chunk loads ahead of the weight constants so the
            # first gathers are not queued behind them on the HWDGE FIFO
            col_off = [0]
            for _, _, sz, _ in schedule:
                col_off.append(col_off[-1] + sz // 8)
            head_cols = col_off[min(4, n_gb)]
            idx_sbA = cpool.tile([128, head_cols], i16, name="idx_sbA")
            nc.sync.dma_start(out=idx_sbA[:], in_=idxT[:, :head_cols])
            ident = cpool.tile([128, 128], f16, name="ident")
            nc.sync.dma_start(out=ident[:], in_=idn[:, :])
            w1_sb = cpool.tile([128, 512], f16, name="w1_sb")
            nc.sync.dma_start(out=w1_sb[:, 0:256], in_=w1[0:128, :])
            nc.sync.dma_start(out=w1_sb[:, 256:512], in_=w1[128:256, :])
            w2_sb = cpool.tile([128, 4], f16, name="w2_sb")
            nc.sync.dma_start(out=w2_sb[:, 0:2], in_=w2[0:128, :])
            nc.sync.dma_start(out=w2_sb[:, 2:4], in_=w2[128:256, :])
            b1_sb = cpool.tile([128, 2], f32, name="b1_sb")
            nc.sync.dma_start(out=b1_sb[:], in_=b1v[:, :])
            b2_sb = cpool.tile([2, 1], f32, name="b2_sb")
            nc.sync.dma_start(out=b2_sb[:], in_=b2v[:, :])
            idx_sbB = cpool.tile([128, max(1, n_idx_cols - head_cols)], i16, name="idx_sbB")
            if n_idx_cols > head_cols:
                nc.sync.dma_start(out=idx_sbB[:], in_=idxT[:, head_cols:])

            for gb, (wl, wr, sz, layoff) in enumerate(schedule):
                nblk = (sz + 127) // 128             # compute chunks (ceil)
                half = sz // 16                      # idx cols per side
                c0 = col_off[gb] - (0 if gb < 4 else head_cols)
                isb = idx_sbA if gb < 4 else idx_sbB
                gl = gpool.tile([128, 8 * CHUNK], f16, name="gl", tag="gl")
                gr = gpool.tile([128, 8 * CHUNK], f16, name="gr", tag="gr")
                nc.gpsimd.dma_gather(
                    out_ap=gl[:, :nblk * D].rearrange("p (b d) -> p b d", d=D),
                    in_ap=embs[wl * W:(wl + 1) * W, :],
                    idxs_ap=isb[:, c0:c0 + half],
                    num_idxs=sz, num_idxs_reg=sz, elem_size=D,
                    queue_num=(2 * gb) % 4,
                )
                nc.gpsimd.dma_gather(
                    out_ap=gr[:, :nblk * D].rearrange("p (b d) -> p b d", d=D),
                    in_ap=embs[wr * W:(wr + 1) * W, :],
                    idxs_ap=isb[:, c0 + half:c0 + 2 * half],
                    num_idxs=sz, num_idxs_reg=sz, elem_size=D,
                    queue_num=(2 * gb + 1) % 4,
                )
                ob = opool.tile([2, GBP], f32, name="ob", tag="ob")
                # v-blocks of up to 512 pairs; the tail block may be any
                # multiple of 128 (block sizes are 128-granular)
                vszs = []
                rem_chunks = nblk
                while rem_chunks > 0:
                    take = min(4, rem_chunks)
                    vszs.append(take * CHUNK)
                    rem_chunks -= take
                n_v = len(vszs)
                voff = [0]
                for vsz in vszs:
                    voff.append(voff[-1] + vsz)
                # transposes for all compute blocks, then W1 matmuls grouped
                # by stationary operand so each W1 chunk is loaded once per GB
                xts = []   # (xtl, xtr) per v
                for v, vsz in enumerate(vszs):
                    xtl_ps = ps_x.tile([128, NB], f16, name="xtl_ps", tag="psx")
                    xtr_ps = ps_x.tile([128, NB], f16, name="xtr_ps", tag="psx")
                    for s in range(vsz // CHUNK):
                        c = voff[v] + s * CHUNK
                        nc.tensor.transpose(
                            out=xtl_ps[:, s * 128:(s + 1) * 128],
                            in_=gl[:, c:c + 128],
                            identity=ident[:],
                        )
                        nc.tensor.transpose(
                            out=xtr_ps[:, s * 128:(s + 1) * 128],
                            in_=gr[:, c:c + 128],
                            identity=ident[:],
                        )
                    xtl = xpool.tile([128, NB], f16, name="xtl", tag="xt")
                    xtr = xpool.tile([128, NB], f16, name="xtr", tag="xt")
                    nc.vector.tensor_copy(xtl[:, :vsz], xtl_ps[:, :vsz])
                    nc.vector.tensor_copy(xtr[:, :vsz], xtr_ps[:, :vsz])
                    xts.append((xtl, xtr))

                hs = [(ps_h.tile([128, NB], f32, name=f"h0v{v}", tag="psh"),
                       ps_h.tile([128, NB], f32, name=f"h1v{v}", tag="psh"))
                      for v in range(n_v)]
                # W1 matmuls grouped by stationary operand across v
                for v, vsz in enumerate(vszs):
                    nc.tensor.matmul(hs[v][0][:, :vsz], w1_sb[:, 0:128],
                                     xts[v][0][:, :vsz], start=True, stop=False)
                for v, vsz in enumerate(vszs):
                    nc.tensor.matmul(hs[v][0][:, :vsz], w1_sb[:, 256:384],
                                     xts[v][1][:, :vsz], start=False, stop=True)
                for v, vsz in enumerate(vszs):
                    nc.tensor.matmul(hs[v][1][:, :vsz], w1_sb[:, 128:256],
                                     xts[v][0][:, :vsz], start=True, stop=False)
                for v, vsz in enumerate(vszs):
                    nc.tensor.matmul(hs[v][1][:, :vsz], w1_sb[:, 384:512],
                                     xts[v][1][:, :vsz], start=False, stop=True)

                for v, vsz in enumerate(vszs):
                    ht0 = hpool.tile([128, NB], f16, name="ht0", tag="ht")
                    ht1 = hpool.tile([128, NB], f16, name="ht1", tag="ht")
                    nc.scalar.activation(ht0[:, :vsz], hs[v][0][:, :vsz], RELU,
                                         bias=b1_sb[:, 0:1], scale=1.0)
                    nc.scalar.activation(ht1[:, :vsz], hs[v][1][:, :vsz], RELU,
                                         bias=b1_sb[:, 1:2], scale=1.0)
                    lps = ps_l.tile([2, NB], f32, name="lps", tag="psl")
                    nc.tensor.matmul(lps[:, :vsz], w2_sb[:, 0:2], ht0[:, :vsz],
                                     start=True, stop=False)
                    nc.tensor.matmul(lps[:, :vsz], w2_sb[:, 2:4], ht1[:, :vsz],
                                     start=False, stop=True)
                    nc.vector.tensor_tensor(
                        out=ob[:, voff[v]:voff[v] + vsz], in0=lps[:, :vsz],
                        in1=b2_sb[:, 0:1].to_broadcast([2, vsz]),
                        op=mybir.AluOpType.add,
                    )
                nc.sync.dma_start(
                    out=outT[:, layoff:layoff + sz], in_=ob[:, :sz],
                )

    nc.compile()
    return nc


def plan_schedule(idx_all_i32):
    """Global plan: bucket ALL pairs by (windowL, windowR), deal each group
    evenly across the 8 cores, pad per-core group sizes to 16-granular
    gathers. Blocks carry their layout offset; the small remainder blocks
    are emitted last (smallest at the very end) so the end-of-stream
    compute backlog drains fast. Returns (schedule, S, bal, key)."""
    key = (idx_all_i32[:, 0] // W) * 4 + (idx_all_i32[:, 1] // W)
    gtot = np.bincount(key, minlength=16)
    bal = (gtot + N_CORES - 1) // N_CORES          # per-core share (ceil)
    S = ((bal + 15) // 16) * 16                    # 16-granular padded size
    starts = np.zeros(17, np.int64)
    starts[1:] = np.cumsum(S)
    fulls, rems = [], []
    for g in range(16):
        full, rem = divmod(int(S[g]), GBP)
        for k in range(full):
            fulls.append((g // 4, g % 4, GBP, int(starts[g]) + k * GBP))
        if rem:
            rems.append((g // 4, g % 4, rem, int(starts[g]) + full * GBP))
    # small remainder blocks go FIRST: early in the stream the gather is the
    # limiter and PE has spare capacity to absorb their lower efficiency;
    # ending on full 1024-blocks keeps the final compute backlog shallow
    rems.sort(key=lambda b: b[2])
    return tuple(rems + fulls), S, bal, key


def prepare_core(share, S, schedule):
    """share: list of 16 arrays of [n_g, 2] node-index pairs (this core's
    share of each group). Builds the wrapped idx tensor; real pairs sit at
    the front of each group's padded span."""
    e_pad = int(S.sum())
    starts = np.zeros(17, np.int64)
    starts[1:] = np.cumsum(S)
    L = np.empty(e_pad, np.int32)
    R = np.empty(e_pad, np.int32)
    for g in range(16):
        L[starts[g]:starts[g + 1]] = (g // 4) * W
        R[starts[g]:starts[g + 1]] = (g % 4) * W
        n = len(share[g])
        L[starts[g]:starts[g] + n] = share[g][:, 0]
        R[starts[g]:starts[g] + n] = share[g][:, 1]
    L16 = (L - (L // W) * W).astype(np.int16)
    R16 = (R - (R // W) * W).astype(np.int16)
    # idx columns follow the schedule's block emission order; each block
    # pulls its pairs from its layout offset
    total_cols = sum(sz // 8 for _, _, sz, _ in schedule)
    cols = np.empty((128, total_cols), np.int16)
    c_off = 0
    for _, _, sz, layoff in schedule:
        for arr in (L16, R16):
            seg = arr[layoff:layoff + sz]
            wt = seg.reshape(sz // 16, 16).T       # [16, sz/16]
            cols[:, c_off:c_off + sz // 16] = np.tile(wt, (8, 1))
            c_off += sz // 16
    return np.ascontiguousarray(cols)


_CACHE = {}


def _get_program(schedule):
    if _CACHE.get("schedule") != schedule:
        _CACHE["nc"] = build_program(schedule)
        _CACHE["schedule"] = schedule
    return _CACHE["nc"]


def run_on_hw(nc, in_maps, trace=False, **kw):
    old = nc.m
    nc.m = get_hw_module(nc.m)
    try:
        return bass_utils.run_bass_kernel_spmd(
            nc, in_maps, core_ids=list(range(len(in_maps))), trace=trace, **kw
        )
    finally:
        nc.m = old


def make_in_maps(spatial_nodes_embs, node_indices, W1, b1, W2, b2):
    embs = np.ascontiguousarray(np.asarray(spatial_nodes_embs), dtype=np.float16)
    idx = np.asarray(node_indices).astype(np.int32)
    w1 = np.ascontiguousarray(np.asarray(W1), dtype=np.float16)
    b1 = np.asarray(b1, dtype=np.float32)
    w2 = np.ascontiguousarray(np.asarray(W2), dtype=np.float16)
    b2 = np.asarray(b2, dtype=np.float32)
    b1v = np.ascontiguousarray(b1.reshape(2, 128).T)
    b2v = np.ascontiguousarray(b2.reshape(2, 1))
    idn = np.eye(128, dtype=np.float16)
    schedule, S, bal, key = plan_schedule(idx)
    starts = np.zeros(17, np.int64)
    starts[1:] = np.cumsum(S)
    group_members = [np.flatnonzero(key == g) for g in range(16)]
    in_maps, origs, poss = [], [], []
    for c in range(N_CORES):
        share, orig, pos = [], [], []
        for g in range(16):
            mem = group_members[g][c * bal[g]:(c + 1) * bal[g]]
            share.append(idx[mem])
            orig.append(mem)
            pos.append(starts[g] + np.arange(len(mem)))
        cols = prepare_core(share, S, schedule)
        origs.append(np.concatenate(orig))
        poss.append(np.concatenate(pos))
        in_maps.append({
            "embs": embs, "idxT": cols, "w1": w1, "b1v": b1v,
            "w2": w2, "b2v": b2v, "idn": idn,
        })
    return schedule, in_maps, origs, poss


def kernel(spatial_nodes_embs, node_indices, W1, b1, W2, b2):
    schedule, in_maps, origs, poss = make_in_maps(
        spatial_nodes_embs, node_indices, W1, b1, W2, b2)
    nc = _get_program(schedule)
    res = run_on_hw(nc, in_maps)
    out = np.empty((E_TOTAL, 2), dtype=np.float32)
    for c in range(N_CORES):
        oT = res.results[c]["outT"]              # [2, e_pad]
        out[origs[c]] = oT[:, poss[c]].T         # scatter to original order
    return out

